# revision 1
# baseline (speedup 1.0000x reference)
"""Trainium2 Bass kernel for nn_CrossAttention (B=16, S=E=1024, H=2048).

Sharding: data-parallel over batch across 8 NeuronCores (2 batches/core).
Math per batch b:
  q = pl @ Wq ; k = sam @ Wk ; v = sam @ Wv
  scores = q @ k^T / sqrt(E)
  w = softmax over the WHOLE flattened [S*S] score matrix  (global max / sum)
  attn = w @ v
  x = LN(attn + pl) * g1 + b1
  out = LN(x @ W1 @ W2 + x) * g2 + b2

On-chip strategy (per core):
  - All matmuls in fp32r (full-rate fp32 on the PE, free-dim 512).
  - Q and K are produced TRANSPOSED (QT/KT = [f, s]) directly from the
    projections, so scores^T = KT^T-contraction needs no transposes, and
    exp(scores^T) = wT is exactly the stationary operand attn needs.
  - Global softmax: per-tile row maxes -> gpsimd partition_all_reduce(max),
    exp via ACT with fused scale (1/32) / bias (-max) and accum_out row
    sums -> partition_all_reduce(add) -> 1/Z folded into attn eviction.
  - LayerNorm via bn_stats/bn_aggr; gamma/beta applied via partition-
    broadcast replicated rows.
  - pl/sam/x transposed with PE-mode transpose (128x128 blocks).
"""

import numpy as np

import concourse.bass as bass
import concourse.bass_isa as bass_isa
import concourse.mybir as mybir
import concourse.tile as tile
from concourse import bacc
from concourse.bass import ts
from concourse.bass_utils import run_bass_kernel_spmd
from concourse.masks import make_identity

F32 = mybir.dt.float32
F32R = mybir.dt.float32r
AF = mybir.ActivationFunctionType
ALU = mybir.AluOpType
AX = mybir.AxisListType

B, S, E, H = 16, 1024, 1024, 2048
NCORES = 8
BPC = B // NCORES  # batches per core
P = 128
NT = S // P      # 8 row-tiles per 1024
NH = H // P      # 16 row-tiles per 2048
NCH = S // 512   # 2 512-chunks per 1024
EPS = 1e-5
SCALE = 1.0 / 32.0  # 1/sqrt(E)


def r(ap):
    """View an fp32 AP as fp32r for full-rate PE matmuls."""
    return ap.bitcast(F32R)


def build_kernel(phase_marks=None):
    def mark(name):
        if phase_marks is not None:
            phase_marks.append((name, _next_id_probe()))

    nc = bacc.Bacc("TRN2", debug=False, num_devices=NCORES)
    def _next_id_probe():
        return nc.next_id()


    pl = nc.dram_tensor("pl", [BPC, S, E], F32, kind="ExternalInput")
    sam = nc.dram_tensor("sam", [BPC, S, E], F32, kind="ExternalInput")
    wq_d = nc.dram_tensor("wq", [E, E], F32R, kind="ExternalInput")
    wk_d = nc.dram_tensor("wk", [E, E], F32R, kind="ExternalInput")
    wv_d = nc.dram_tensor("wv", [E, E], F32R, kind="ExternalInput")
    g1_d = nc.dram_tensor("g1", [E], F32, kind="ExternalInput")
    b1_d = nc.dram_tensor("b1", [E], F32, kind="ExternalInput")
    w1_d = nc.dram_tensor("w1", [E, H], F32R, kind="ExternalInput")
    w2_d = nc.dram_tensor("w2", [H, E], F32R, kind="ExternalInput")
    g2_d = nc.dram_tensor("g2", [E], F32, kind="ExternalInput")
    b2_d = nc.dram_tensor("b2", [E], F32, kind="ExternalInput")
    out = nc.dram_tensor("out", [BPC, S, E], F32, kind="ExternalOutput")

    def bcast_row(handle):
        """DRAM [E] -> AP broadcasting along the partition dim: [128, E]."""
        ap = handle.ap()
        return bass.AP(tensor=ap.tensor, offset=ap.offset, ap=[[0, P], ap.ap[0]])

    with tile.TileContext(nc) as tc:
        consts = tc.alloc_tile_pool(name="consts", bufs=1)
        big = tc.alloc_tile_pool(name="big", bufs=1)
        streams = tc.alloc_tile_pool(name="streams", bufs=2)
        stats = tc.alloc_tile_pool(name="stats", bufs=10)
        psum = tc.alloc_tile_pool(name="psum", bufs=6, space="PSUM")
        psumt = tc.alloc_tile_pool(name="psumt", bufs=2, space="PSUM")

        ident = consts.tile([P, P], F32)
        make_identity(nc, ident)
        g1r = consts.tile([P, E], F32)
        b1r = consts.tile([P, E], F32)
        g2r = consts.tile([P, E], F32)
        b2r = consts.tile([P, E], F32)
        nc.gpsimd.dma_start(out=g1r, in_=bcast_row(g1_d))
        nc.gpsimd.dma_start(out=b1r, in_=bcast_row(b1_d))
        nc.gpsimd.dma_start(out=g2r, in_=bcast_row(g2_d))
        nc.gpsimd.dma_start(out=b2r, in_=bcast_row(b2_d))
        epst = consts.tile([P, 1], F32)
        nc.vector.memset(epst, EPS)

        # 5 rotating 32KB/partition slots; lifetimes are disjoint per color.
        def slot(name, c, shape=(P, NT, S), dtype=F32R):
            return big.tile(list(shape), dtype, tag=f"c{c}", name=name)

        def transpose_from_dram(dst, src3d, b, evict_engine):
            """dst[p, j, i*128:(i+1)*128] = src[b, i*128+p? ...]^T blocks."""
            for i in range(NT):
                nat = streams.tile([P, S], F32, tag="nat", bufs=4, name=f"nat_{b}_{i}")
                nc.sync.dma_start(out=nat, in_=src3d[b, ts(i, P), :])
                for j in range(NT):
                    pst = psumt.tile([P, P], F32, tag="tp", name=f"tp_{b}_{i}_{j}")
                    nc.tensor.transpose(pst, nat[:, ts(j, P)], ident)
                    if evict_engine == "act":
                        nc.scalar.copy(out=dst[:, j, ts(i, P)], in_=pst)
                    else:
                        nc.vector.tensor_copy(out=dst[:, j, ts(i, P)], in_=pst)

        for b in range(BPC):
            # SBUF slot coloring (4 x 32KB/partition, disjoint lifetimes):
            #   c0: plT -> KT -> xT -> w2h0/w2h1
            #   c1: samT -> sc/wT -> hT0
            #   c2: QT -> rx (r -> x -> ff+x -> out, in place)
            #   c3: V -> hT1
            # ---- Phase A: transposed loads ----------------------------------
            mark("A0_start_b{}".format(b))
            plT = slot(f"plT_{b}", 0)
            transpose_from_dram(plT, pl, b, "act")
            samT = slot(f"samT_{b}", 1)
            transpose_from_dram(samT, sam, b, "vec")

            # ---- Phase B: projections  QT, KT (transposed), V (natural) ----
            mark("A_transposes_done_b{}".format(b))
            QT = slot(f"QT_{b}", 2)
            KT = slot(f"KT_{b}", 0)
            for name, WD, src, dst in (("q", wq_d, plT, QT), ("k", wk_d, samT, KT)):
                for f in range(NT):
                    wcol = streams.tile([P, NT, P], F32R, tag="wcol",
                                        name=f"wcol_{name}_{b}_{f}")
                    nc.sync.dma_start(
                        out=wcol,
                        in_=WD[:, ts(f, P)].rearrange("(t p) c -> p t c", p=P))
                    for ch in range(NCH):
                        ps = psum.tile([P, 512], F32, tag="mm",
                                       name=f"ps{name}_{b}_{f}_{ch}")
                        for e_t in range(NT):
                            nc.tensor.matmul(
                                ps, r(wcol[:, e_t, :]),
                                r(src[:, e_t, ts(ch, 512)]),
                                start=(e_t == 0), stop=(e_t == NT - 1))
                        nc.scalar.copy(out=dst[:, f, ts(ch, 512)], in_=ps)

            # V natural: V[t,e'] = sum_e sam[t,e] Wv[e,e'];
            # lhsT = samT[e_tile, t_block] (stationary), rhs = Wv rows.
            # Per output half-column ch, keep Wv[:, ch] resident (16KB/part).
            mark("B_qk_done_b{}".format(b))
            V = slot(f"V_{b}", 3)
            for ch in range(NCH):
                wvh = streams.tile([P, NT, 512], F32R, tag="wbig",
                                   bufs=1, name=f"wvh_{b}_{ch}")
                for k in range(NT):
                    nc.sync.dma_start(
                        out=wvh[:, k, :], in_=wv_d[ts(k, P), ts(ch, 512)])
                for t in range(NT):
                    ps = psum.tile([P, 512], F32, tag="mm", name=f"psv_{b}_{t}_{ch}")
                    for e_t in range(NT):
                        nc.tensor.matmul(
                            ps, r(samT[:, e_t, ts(t, P)]),
                            r(wvh[:, e_t, :]),
                            start=(e_t == 0), stop=(e_t == NT - 1))
                    nc.scalar.copy(out=V[:, t, ts(ch, 512)], in_=ps)

            # ---- Phase C: scores^T + flattened softmax ----------------------
            mark("B_v_done_b{}".format(b))
            sc = slot(f"sc_{b}", 1)  # raw scores^T, becomes wT after exp
            mx = stats.tile([P, NT * NCH], F32, tag="sm", name=f"mx_{b}")
            for t in range(NT):
                for ch in range(NCH):
                    ps = psum.tile([P, 512], F32, tag="mm", name=f"pss_{b}_{t}_{ch}")
                    for f in range(NT):
                        nc.tensor.matmul(
                            ps, r(KT[:, f, ts(t, P)]), r(QT[:, f, ts(ch, 512)]),
                            start=(f == 0), stop=(f == NT - 1))
                    nc.vector.tensor_copy(out=sc[:, t, ts(ch, 512)], in_=ps)
                    nc.vector.tensor_reduce(
                        out=mx[:, t * NCH + ch : t * NCH + ch + 1],
                        in_=sc.bitcast(F32)[:, t, ts(ch, 512)], axis=AX.X, op=ALU.max)

            mark("C_scores_done_b{}".format(b))

            mxr = stats.tile([P, 1], F32, tag="sm", name=f"mxr_{b}")
            nc.vector.tensor_reduce(out=mxr, in_=mx, axis=AX.X, op=ALU.max)
            mall = stats.tile([P, 1], F32, tag="sm", name=f"mall_{b}")
            nc.gpsimd.partition_all_reduce(
                out_ap=mall, in_ap=mxr, channels=P, reduce_op=bass_isa.ReduceOp.max)
            ebias = stats.tile([P, 1], F32, tag="sm", name=f"ebias_{b}")
            nc.vector.tensor_scalar_mul(out=ebias, in0=mall, scalar1=-SCALE)
            rows = stats.tile([P, NT * NCH], F32, tag="sm", name=f"rows_{b}")
            for ch in range(NCH):
                for t in range(NT):
                    nc.scalar.activation(
                        out=sc[:, t, ts(ch, 512)],
                        in_=sc.bitcast(F32)[:, t, ts(ch, 512)], func=AF.Exp,
                        bias=ebias, scale=SCALE,
                        accum_out=rows[:, t * NCH + ch : t * NCH + ch + 1])
            zp = stats.tile([P, 1], F32, tag="sm", name=f"zp_{b}")
            nc.vector.tensor_reduce(out=zp, in_=rows, axis=AX.X, op=ALU.add)
            ztot = stats.tile([P, 1], F32, tag="sm", name=f"ztot_{b}")
            nc.gpsimd.partition_all_reduce(
                out_ap=ztot, in_ap=zp, channels=P, reduce_op=bass_isa.ReduceOp.add)
            zinv = stats.tile([P, 1], F32, tag="sm", name=f"zinv_{b}")
            nc.vector.reciprocal(out=zinv, in_=ztot)

            # ---- Phase D: attn = (wT^T @ V) / Z + pl ; LN1 -> x -------------
            mark("C_softmax_done_b{}".format(b))
            rx = slot(f"rx_{b}", 2, dtype=F32)  # r = attn/Z + pl, then x in place
            for st in range(NT):
                nat = streams.tile([P, S], F32, tag="nat", bufs=4, name=f"natr_{b}_{st}")
                nc.sync.dma_start(out=nat, in_=pl[b, ts(st, P), :])
                for ch in range(NCH):
                    ps = psum.tile([P, 512], F32, tag="mm", name=f"psa_{b}_{st}_{ch}")
                    for t in range(NT):
                        nc.tensor.matmul(
                            ps, r(sc[:, t, ts(st, P)]), r(V[:, t, ts(ch, 512)]),
                            start=(t == 0), stop=(t == NT - 1))
                    nc.vector.scalar_tensor_tensor(
                        out=rx[:, st, ts(ch, 512)], in0=ps, scalar=zinv,
                        in1=nat[:, ts(ch, 512)], op0=ALU.mult, op1=ALU.add)

                # LayerNorm 1 on rows of rx[:, st, :]
                bst = stats.tile([P, 2, 6], F32, tag="ln", name=f"bst1_{b}_{st}")
                for h in range(2):
                    nc.vector.bn_stats(out=bst[:, h, :], in_=rx[:, st, ts(h, 512)])
                mv = stats.tile([P, 2], F32, tag="ln", name=f"mv1_{b}_{st}")
                nc.vector.bn_aggr(out=mv, in_=bst)
                sd = stats.tile([P, 1], F32, tag="ln", name=f"sd1_{b}_{st}")
                nc.scalar.activation(out=sd, in_=mv[:, 1:2], func=AF.Sqrt, bias=epst)
                rstd = stats.tile([P, 1], F32, tag="ln", name=f"rstd1_{b}_{st}")
                nc.vector.reciprocal(out=rstd, in_=sd)
                nc.vector.tensor_scalar(
                    out=rx[:, st, :], in0=rx[:, st, :],
                    scalar1=mv[:, 0:1], scalar2=rstd,
                    op0=ALU.subtract, op1=ALU.mult)
                nc.vector.scalar_tensor_tensor(
                    out=rx[:, st, :], in0=rx[:, st, :], scalar=0.0, in1=g1r,
                    op0=ALU.add, op1=ALU.mult)
                nc.gpsimd.tensor_add(out=rx[:, st, :], in0=rx[:, st, :], in1=b1r)

            # ---- Phase E: xT, hT = (x @ W1)^T -------------------------------
            mark("D_attn_ln1_done_b{}".format(b))
            xT = slot(f"xT_{b}", 0)
            for i in range(NT):
                for j in range(NT):
                    pst = psumt.tile([P, P], F32, tag="tp", name=f"tpx_{b}_{i}_{j}")
                    nc.tensor.transpose(pst, rx[:, i, ts(j, P)], ident)
                    nc.scalar.copy(out=xT[:, j, ts(i, P)], in_=pst)

            mark("E_xT_done_b{}".format(b))
            hT = [slot(f"hT0_{b}", 1), slot(f"hT1_{b}", 3)]
            for ht in range(NH):
                wcol = streams.tile([P, NT, P], F32R, tag="wcol",
                                    name=f"wcol1_{b}_{ht}")
                nc.sync.dma_start(
                    out=wcol, in_=w1_d[:, ts(ht, P)].rearrange("(t p) c -> p t c", p=P))
                for ch in range(NCH):
                    ps = psum.tile([P, 512], F32, tag="mm", name=f"psh_{b}_{ht}_{ch}")
                    for e_t in range(NT):
                        nc.tensor.matmul(
                            ps, r(wcol[:, e_t, :]), r(xT[:, e_t, ts(ch, 512)]),
                            start=(e_t == 0), stop=(e_t == NT - 1))
                    nc.scalar.copy(
                        out=hT[ht // NT][:, ht % NT, ts(ch, 512)], in_=ps)

            # ---- Phase F: ff = hT^T @ W2 + x (in place), LN2, store ---------
            mark("E_hT_done_b{}".format(b))
            # Two half-contractions; each W2 half lives in a dedicated stream
            # slot and is prefetched in 8 per-k-tile DMA chunks so the ff
            # matmuls stream behind the DMA instead of stalling on one 4MB
            # transfer (which also re-throttled the PE clock).
            for half in range(2):
                w2h = streams.tile([P, NT, S], F32R, tag="wbig",
                                   bufs=1, name=f"w2h{half}_{b}")
                for k in range(NT):
                    nc.sync.dma_start(
                        out=w2h[:, k, :],
                        in_=w2_d[ts(half * NT + k, P), :])
                for st in range(NT):
                    for ch in range(NCH):
                        ps = psum.tile([P, 512], F32, tag="mm",
                                       name=f"psf_{b}_{half}_{st}_{ch}")
                        for k in range(NT):
                            nc.tensor.matmul(
                                ps, r(hT[half][:, k, ts(st, P)]),
                                r(w2h[:, k, ts(ch, 512)]),
                                start=(k == 0), stop=(k == NT - 1))
                        # rx[:, st, ch] += ps   (residual on half 0 adds to x)
                        nc.vector.scalar_tensor_tensor(
                            out=rx[:, st, ts(ch, 512)], in0=ps, scalar=0.0,
                            in1=rx[:, st, ts(ch, 512)], op0=ALU.add, op1=ALU.add)

                    if half == 1:
                        # LayerNorm 2 (in place on rx) and store, interleaved
                        # with the remaining ff matmuls.
                        bst = stats.tile([P, 2, 6], F32, tag="ln",
                                         name=f"bst2_{b}_{st}")
                        for h in range(2):
                            nc.vector.bn_stats(out=bst[:, h, :],
                                               in_=rx[:, st, ts(h, 512)])
                        mv = stats.tile([P, 2], F32, tag="ln", name=f"mv2_{b}_{st}")
                        nc.vector.bn_aggr(out=mv, in_=bst)
                        sd = stats.tile([P, 1], F32, tag="ln", name=f"sd2_{b}_{st}")
                        nc.scalar.activation(out=sd, in_=mv[:, 1:2],
                                             func=AF.Sqrt, bias=epst)
                        rstd = stats.tile([P, 1], F32, tag="ln",
                                          name=f"rstd2_{b}_{st}")
                        nc.vector.reciprocal(out=rstd, in_=sd)
                        nc.vector.tensor_scalar(
                            out=rx[:, st, :], in0=rx[:, st, :],
                            scalar1=mv[:, 0:1], scalar2=rstd,
                            op0=ALU.subtract, op1=ALU.mult)
                        nc.vector.scalar_tensor_tensor(
                            out=rx[:, st, :], in0=rx[:, st, :], scalar=0.0,
                            in1=g2r, op0=ALU.add, op1=ALU.mult)
                        nc.gpsimd.tensor_add(out=rx[:, st, :],
                                             in0=rx[:, st, :], in1=b2r)
                        nc.sync.dma_start(out=out[b, ts(st, P), :],
                                          in_=rx[:, st, :])
            mark("F_ff_done_b{}".format(b))

        psumt.release()
        psum.release()
        stats.release()
        streams.release()
        big.release()
        consts.release()

    nc.finalize()
    return nc


_NC_CACHE = None


def _get_nc():
    global _NC_CACHE
    if _NC_CACHE is None:
        _NC_CACHE = build_kernel()
    return _NC_CACHE


def make_in_maps(ins):
    common = {
        "wq": np.ascontiguousarray(ins["Wq"], np.float32),
        "wk": np.ascontiguousarray(ins["Wk"], np.float32),
        "wv": np.ascontiguousarray(ins["Wv"], np.float32),
        "g1": np.ascontiguousarray(ins["ln1_g"], np.float32),
        "b1": np.ascontiguousarray(ins["ln1_b"], np.float32),
        "w1": np.ascontiguousarray(ins["W1"], np.float32),
        "w2": np.ascontiguousarray(ins["W2"], np.float32),
        "g2": np.ascontiguousarray(ins["ln2_g"], np.float32),
        "b2": np.ascontiguousarray(ins["ln2_b"], np.float32),
    }
    in_maps = []
    for c in range(NCORES):
        m = dict(common)
        m["pl"] = np.ascontiguousarray(
            ins["pl_source"][c * BPC : (c + 1) * BPC], np.float32)
        m["sam"] = np.ascontiguousarray(
            ins["sam_source"][c * BPC : (c + 1) * BPC], np.float32)
        in_maps.append(m)
    return in_maps


def kernel(pl_source, sam_source, Wq, Wk, Wv, ln1_g, ln1_b, W1, W2, ln2_g, ln2_b):
    nc = _get_nc()
    in_maps = make_in_maps({
        "pl_source": pl_source, "sam_source": sam_source,
        "Wq": Wq, "Wk": Wk, "Wv": Wv, "ln1_g": ln1_g, "ln1_b": ln1_b,
        "W1": W1, "W2": W2, "ln2_g": ln2_g, "ln2_b": ln2_b,
    })
    res = run_bass_kernel_spmd(nc, in_maps, core_ids=list(range(NCORES)))
    return np.concatenate([res.results[c]["out"] for c in range(NCORES)], axis=0)

